# revision 52
# baseline (speedup 1.0000x reference)
"""Trainium2 Bass kernel for nn_BaseX2HAttLayer (GNN edge-attention layer).

Strategy (v2)
-------------
Host: stable-sort edges by dst. Pad nodes to 10240 = 8 cores x 10 blocks x
128. Each core owns 1280 contiguous dst nodes and their edges (softmax
segments never cross cores -> no collectives). Edges per 128-node block are
padded to eb = tiles*128 with tiles a multiple of 4; pad edges get dst
slot -1 so their one-hot row is all-zero and they contribute nothing.

Key algebraic moves (all pure weight reparameterizations done on host):
  * W1 columns of the edge MLPs (hk, hv) are centered per 128-wide hidden
    half, so the LayerNorm mean is identically zero -> the per-tile
    scale/bias activation collapses to one uniform wide ReLU per PSUM bank.
  * relu(r*x) = r*relu(x) for r>0: the LN inv-std is applied AFTER the
    second linear layer - on the k side it multiplies the logits (one wide
    tensor op), on the v side it rides along with the e_w sigmoid factor.
  * b2 biases of hk/hv are zero for this problem (asserted; falls back to
    the v1 program otherwise).

Device per core: per 128-node block, edges are processed in quads (4 tiles
of 128 edges). PSUM: 4 rotating z-banks (pairs of tiles; reused for the
second-layer outputs), 2 q-gather banks, 1 transpose bank, 1 segment
accumulator (+ e_w logit slack). The tensor engine is kept continuously
busy (HAM warm state) by emitting quad q's L1 matmuls ahead of quad q-1's
tail. bn_stats is grouped per bank; softmax/e_w/logit work is spread
across Vector / Scalar / GPSIMD.
"""

import os
import sys

sys.path.insert(0, "/opt/trn_rl_repo")

import ml_dtypes
import numpy as np

import concourse.bass as bass
import concourse.mybir as mybir
from concourse.bass_utils import run_bass_kernel_spmd
from concourse.tile import TileContext

F32 = mybir.dt.float32
BF16 = mybir.dt.bfloat16
AF = mybir.ActivationFunctionType
OP = mybir.AluOpType
NPBF = ml_dtypes.bfloat16

N, E = 10000, 320000
DIM = 128
NH, HD = 16, 8
EFD, RFD = 4, 64
REF = EFD + RFD  # 68
NCORES = 8
NPAD = 10240
NPC = NPAD // NCORES  # 1280
NBLK = NPC // 128  # 10
LN_EPS = 1e-5
DEN_EPS = 1e-16
RS8 = float(1.0 / np.sqrt(HD))


def _bf(ap):
    return ap.bitcast(BF16)


def _sap(tile, base_ap, offset_ap, dims):
    """Manual AP on tile with explicit [stride, count] free dims."""
    return bass.AP(tensor=tile.tensor, offset=offset_ap.offset,
                   ap=[base_ap.ap[0]] + dims)


# ---------------------------------------------------------------------------
# compile-path workarounds (this image)
# ---------------------------------------------------------------------------


def _split_multiwait_drains(nc):
    """This walrus build allows few sync-waits per instruction. Hoist excess
    waits onto single-wait Drains inserted just before, on the same engine."""
    ctr = [0]
    for fn in nc.m.functions:
        for bb in fn.blocks:
            out = []
            for ins in bb.instructions:
                si = ins.sync_info
                limit = 1
                if si is not None and len(si.on_wait) > limit:
                    waits = list(si.on_wait)
                    for w in waits[:-limit]:
                        d = mybir.InstDrain(
                            name=f"I-splitw-{ctr[0]}", ins=[], outs=[]
                        )
                        ctr[0] += 1
                        d.engine = ins.engine
                        d.sync_info = mybir.SyncInfo(on_wait=[w], on_update=[])
                        nc.register_instruction(d, overwrite=True)
                        out.append(d)
                    ins.sync_info = mybir.SyncInfo(
                        on_wait=waits[-limit:], on_update=list(si.on_update)
                    )
                out.append(ins)
            bb.instructions[:] = out


def _install_ntff_hook_shim():
    import types

    if "antenv.axon_hooks" in sys.modules:
        return
    import antenv

    mod = types.ModuleType("antenv.axon_hooks")
    state = {"hook": None, "init": False}

    def set_axon_ntff_profile_hook(hook):
        state["hook"] = hook
        state["init"] = True

    def get_axon_ntff_profile_hook():
        if not state["init"]:
            try:
                from trn_agent_boot.trn_boot import _ntff_profile_via_ctypes

                state["hook"] = _ntff_profile_via_ctypes(
                    "/opt/axon/libaxon_pjrt.so"
                )
            except Exception:
                state["hook"] = None
            state["init"] = True
        return state["hook"]

    mod.set_axon_ntff_profile_hook = set_axon_ntff_profile_hook
    mod.get_axon_ntff_profile_hook = get_axon_ntff_profile_hook
    sys.modules["antenv.axon_hooks"] = mod
    antenv.axon_hooks = mod


# ---------------------------------------------------------------------------
# host-side prep (v2)
# ---------------------------------------------------------------------------


def _prep_inputs_v2(inputs):
    h = np.asarray(inputs["h"], np.float32)
    r_feat = np.asarray(inputs["r_feat"], np.float32)
    edge_feat = np.asarray(inputs["edge_feat"], np.float32)
    ei = np.asarray(inputs["edge_index"])
    src, dst = ei[0].astype(np.int64), ei[1].astype(np.int64)

    order = np.argsort(dst, kind="stable")
    src_s, dst_s = src[order], dst[order]
    ref_s = np.concatenate([edge_feat[order], r_feat[order]], axis=1)  # [E,68]
    ew_W = np.asarray(inputs["ew_W"], np.float32)
    u_s = (r_feat @ ew_W)[:, 0][order]  # e_w gate logit (pre-bias)

    nblk_tot = NPAD // 128
    starts = np.searchsorted(dst_s, np.arange(nblk_tot) * 128)
    ends = np.searchsorted(dst_s, (np.arange(nblk_tot) + 1) * 128)
    cnts = ends - starts
    tiles = int((cnts.max() + 127) // 128)
    tiles = ((tiles + 3) // 4) * 4
    eb = tiles * 128

    hpad = np.zeros((NPAD, DIM), np.float32)
    hpad[:N] = h

    per_core = []
    for c in range(NCORES):
        reT = np.zeros((REF, NBLK * eb), np.float32)
        hjT = np.zeros((DIM, NBLK * eb), np.float32)
        dstloc = np.full((128, NBLK * tiles), -1.0, np.float32)
        ulog = np.zeros((128, NBLK * tiles), np.float32)
        dstT = np.full((NBLK * eb,), -1.0, np.float32)
        for b in range(NBLK):
            g = c * NBLK + b
            s0, cnt = starts[g], cnts[g]
            sl = slice(s0, s0 + cnt)
            reT[:, b * eb : b * eb + cnt] = ref_s[sl].T
            hjT[:, b * eb : b * eb + cnt] = hpad[src_s[sl]].T
            dl = np.full(eb, -1.0, np.float32)
            dl[:cnt] = (dst_s[sl] - g * 128).astype(np.float32)
            dstloc[:, b * tiles : (b + 1) * tiles] = dl.reshape(tiles, 128).T
            dstT[b * eb : b * eb + cnt] = dl[:cnt]
            ul = np.zeros(eb, np.float32)
            ul[:cnt] = u_s[sl]
            ulog[:, b * tiles : (b + 1) * tiles] = ul.reshape(tiles, 128).T
        dstbcT = np.broadcast_to(
            dstT.astype(NPBF)[None, :], (128, NBLK * eb)).copy()
        hrows = np.zeros((128, NBLK, DIM), np.float32)
        blkn = hpad[c * NPC : (c + 1) * NPC].reshape(NBLK, 128, DIM)
        hrows[:, :, :] = blkn.transpose(1, 0, 2)
        hTc = np.ascontiguousarray(hpad[c * NPC : (c + 1) * NPC].T).astype(NPBF)
        per_core.append(
            {"reT": reT.astype(NPBF), "hjT": hjT.astype(NPBF),
             "dstloc": dstloc, "ulog": ulog, "dstbcT": dstbcT,
             "hrows": hrows, "hTc": hTc}
        )
    return per_core, eb


def _center_cols(W):
    """Subtract per-row mean over output columns (makes LN mean exactly 0)."""
    return W - W.mean(axis=1, keepdims=True)


def _prep_weights_v2(inputs):
    g = {k: np.asarray(v, np.float32) for k, v in inputs.items()
         if k != "edge_index"}
    for nm in ("hk", "hv", "hq", "no"):
        assert np.allclose(g[f"{nm}_g1"], 1.0) and np.allclose(
            g[f"{nm}_be1"], 0.0
        ), "LN affine folding requires g1=1, be1=0"

    ok = (not np.any(g["hk_b2"] != 0.0)) and (not np.any(g["hv_b2"] != 0.0))
    if not ok:
        return None  # caller falls back to v1 program

    w = {}
    kW1 = _center_cols(g["hk_W1"])  # [324,128]
    vW1 = _center_cols(g["hv_W1"])
    w["wre"] = np.concatenate([kW1[:REF], vW1[:REF]], axis=1)  # [68,256]
    w["whi"] = np.concatenate(
        [kW1[REF : REF + DIM], vW1[REF : REF + DIM]], axis=1)  # [128,256]
    w["whj"] = np.concatenate([kW1[REF + DIM :], vW1[REF + DIM :]], 1)
    b1k = g["hk_b1"] - g["hk_b1"].mean()
    b1v = g["hv_b1"] - g["hv_b1"].mean()
    w["b1c"] = np.concatenate([b1k, b1v])[None, :]  # [1,256]
    w["w2k"] = g["hk_W2"]
    w["w2v"] = g["hv_W2"]
    w["ewWn"] = np.zeros((REF, 1), np.float32)
    w["ewWn"][EFD:, 0] = -g["ew_W"][:, 0]
    w["ewb"] = float(g["ew_b"][0])
    # q-MLP (computed with full LN on device in phase 1)
    w["wq1"] = g["hq_W1"]
    w["bq1"] = g["hq_b1"][None]
    w["wq2"] = g["hq_W2"]
    w["bq2"] = g["hq_b2"][None]
    # out-MLP
    w["wno1a"] = g["no_W1"][:DIM]
    w["wno1h"] = g["no_W1"][DIM:]
    w["bno1"] = g["no_b1"][None]
    w["wno2"] = g["no_W2"]
    w["bno2"] = g["no_b2"][None]
    w["iota"] = np.broadcast_to(
        np.arange(128, dtype=np.float32), (128, 128)).copy()  # row-arange
    w["iotac"] = np.arange(128, dtype=np.float32)[:, None]  # [128,1]
    w["ident"] = np.eye(128, dtype=np.float32)
    return w


WT_SHAPES_V2 = {
    "wre": ((REF, 256), BF16), "whi": ((DIM, 256), BF16),
    "whj": ((DIM, 256), BF16), "b1c": ((1, 256), BF16),
    "w2k": ((DIM, DIM), BF16), "w2v": ((DIM, DIM), BF16),
    "ewWn": ((REF, 1), BF16),
    "wq1": ((DIM, DIM), BF16), "bq1": ((1, DIM), BF16),
    "wq2": ((DIM, DIM), BF16), "bq2": ((1, DIM), BF16),
    "wno1a": ((DIM, DIM), BF16), "wno1h": ((DIM, DIM), BF16),
    "bno1": ((1, DIM), BF16), "wno2": ((DIM, DIM), BF16),
    "bno2": ((1, DIM), BF16),
    "iota": ((128, 128), BF16), "iotac": ((128, 1), F32),
    "ident": ((128, 128), BF16),
}


# ---------------------------------------------------------------------------
# device program (v2)
# ---------------------------------------------------------------------------


def _ln_chain(nc, wk, psum_src, nhalves, name, eps_ap):
    stats = wk.tile([128, nhalves, 6], F32, tag=f"st{name}")
    mv = wk.tile([128, nhalves, 2], F32, tag=f"mv{name}")
    for hh in range(nhalves):
        nc.vector.bn_stats(out=stats[:, hh, :], in_=psum_src[:, hh, :])
        nc.vector.bn_aggr(out=mv[:, hh, :], in_=stats[:, hh, :])
    lnv = wk.tile([128, nhalves], F32, tag=f"lnv{name}")
    nc.scalar.activation(out=lnv[:, :], in_=mv[:, :, 1], func=AF.Ln,
                         bias=eps_ap, scale=1.0)
    rstd = wk.tile([128, nhalves], F32, tag=f"rstd{name}")
    nc.scalar.activation(out=rstd[:, :], in_=lnv[:, :], func=AF.Exp,
                         bias=0.0, scale=-0.5)
    negmu = wk.tile([128, nhalves], F32, tag=f"ngm{name}")
    nc.vector.tensor_scalar(out=negmu[:, :], in0=mv[:, :, 0], scalar1=-1.0,
                            scalar2=None, op0=OP.mult)
    nmr = wk.tile([128, nhalves], F32, tag=f"nmr{name}")
    nc.vector.tensor_tensor(out=nmr[:, :], in0=negmu[:, :], in1=rstd[:, :],
                            op=OP.mult)
    return rstd, nmr


def build_program_v2(eb, ewb):
    tiles = eb // 128
    nq = tiles // 4
    nc = bass.Bass()

    inp = {}
    inp["reT"] = nc.declare_dram_parameter("reT", [REF, NBLK * eb], BF16,
                                           isOutput=False)
    inp["hjT"] = nc.declare_dram_parameter("hjT", [DIM, NBLK * eb], BF16,
                                           isOutput=False)
    inp["dstloc"] = nc.declare_dram_parameter("dstloc", [128, NBLK * tiles],
                                              F32, isOutput=False)
    inp["ulog"] = nc.declare_dram_parameter("ulog", [128, NBLK * tiles],
                                            F32, isOutput=False)
    inp["dstbcT"] = nc.declare_dram_parameter("dstbcT", [128, NBLK * eb],
                                              BF16, isOutput=False)
    inp["hTc"] = nc.declare_dram_parameter("hTc", [128, NBLK * 128], BF16,
                                           isOutput=False)
    inp["hrows"] = nc.declare_dram_parameter("hrows", [128, NBLK, DIM], F32,
                                             isOutput=False)
    for k, (shp, dt) in WT_SHAPES_V2.items():
        inp[k] = nc.declare_dram_parameter(k, list(shp), dt, isOutput=False)
    out_d = nc.declare_dram_parameter("out", [NPC, DIM], F32, isOutput=True)

    with TileContext(nc, num_cores=NCORES) as tc:
        from contextlib import ExitStack

        with ExitStack() as ctx:
            sg = ctx.enter_context(tc.tile_pool(name="singles", bufs=1))

            # phase-1-critical weights first so the PE can start promptly
            _ord = ["whi", "b1c", "wq1", "bq1", "ident", "wq2", "bq2"]
            _ord += [k for k in WT_SHAPES_V2 if k not in _ord]
            wt = {}
            for k in _ord:
                shp, dt = WT_SHAPES_V2[k]
                wt[k] = sg.tile(list(shp), dt, name=f"wt_{k}", tag=f"wt_{k}")
                nc.sync.dma_start(out=wt[k][:, :], in_=inp[k][:, :])
            ones1 = sg.tile([1, 128], BF16)
            nc.vector.memset(ones1, 1.0)
            epsc = sg.tile([128, 1], F32)
            nc.vector.memset(epsc, LN_EPS)
            ewbc = sg.tile([128, 1], F32)
            nc.vector.memset(ewbc, -ewb)
            hTc = sg.tile([128, NBLK * 128], BF16)
            for k in range(5):
                nc.sync.dma_start(
                    out=hTc[:, k * 256 : (k + 1) * 256],
                    in_=inp["hTc"][:, k * 256 : (k + 1) * 256])
            dstloc = sg.tile([128, NBLK * tiles], F32)
            h5 = NBLK * tiles // 5
            for k in range(5):
                nc.sync.dma_start(
                    out=dstloc[:, k * h5 : (k + 1) * h5],
                    in_=inp["dstloc"][:, k * h5 : (k + 1) * h5])
            ulog = sg.tile([128, NBLK * tiles], F32)
            for k in range(5):
                nc.sync.dma_start(
                    out=ulog[:, k * h5 : (k + 1) * h5],
                    in_=inp["ulog"][:, k * h5 : (k + 1) * h5])
            hrows = sg.tile([128, NBLK, DIM], F32)
            for k in range(5):
                nc.sync.dma_start(
                    out=hrows[:, k * 2 : (k + 1) * 2, :],
                    in_=inp["hrows"][:, k * 2 : (k + 1) * 2, :])
            atab = sg.tile([128, NBLK, 256], BF16)
            qtab = sg.tile([128, NBLK, 128], BF16)

            # --- phase 1: atab (centered hi-part of L1) and q table --------
            with ExitStack() as pre:
                pp = pre.enter_context(
                    tc.tile_pool(name="prepsum", bufs=2, space="PSUM"))
                pw = pre.enter_context(tc.tile_pool(name="prework", bufs=4))

                pst = [None] * NBLK
                for b in range(NBLK + 1):
                    if b < NBLK:
                        hTb = hTc[:, b * 128 : (b + 1) * 128]
                        ps = pp.tile([128, 256], F32, tag="Ap")
                        nc.tensor.matmul(ps[:, :], hTb, wt["whi"][:, :],
                                         start=True, stop=False)
                        nc.tensor.matmul(ps[:, :], ones1[:, :],
                                         wt["b1c"][:, :],
                                         start=False, stop=True)
                        p1 = pp.tile([128, 128], F32, tag="q1")
                        nc.tensor.matmul(p1[:, :], hTb, wt["wq1"][:, :],
                                         start=True, stop=False)
                        nc.tensor.matmul(p1[:, :], ones1[:, :],
                                         wt["bq1"][:, :],
                                         start=False, stop=True)
                        pst[b] = (ps, p1)
                    if b == 0:
                        continue
                    ps, p1 = pst[b - 1]
                    pst[b - 1] = None
                    nc.scalar.copy(out=atab[:, b - 1, :], in_=ps[:, :])
                    rstd, nmr = _ln_chain(
                        nc, pw, p1[:, :].rearrange("p (o f) -> p o f", o=1),
                        1, "q", epsc[:, 0:1])
                    yq = pw.tile([128, 128], BF16, tag="yq")
                    nc.scalar.activation(out=yq[:, :], in_=p1[:, :],
                                         func=AF.Relu, scale=rstd[:, 0:1],
                                         bias=nmr[:, 0:1])
                    pt = pp.tile([128, 64], F32, tag="qT")
                    nc.tensor.transpose(_bf(pt[:, :]), yq[:, :],
                                        wt["ident"][:, :])
                    yqT = pw.tile([128, 128], BF16, tag="yqT")
                    nc.vector.tensor_copy(out=yqT[:, :], in_=_bf(pt[:, :]))
                    p2 = pp.tile([128, 128], F32, tag="q2")
                    nc.tensor.matmul(p2[:, :], yqT[:, :], wt["wq2"][:, :],
                                     start=True, stop=False)
                    nc.tensor.matmul(p2[:, :], ones1[:, :], wt["bq2"][:, :],
                                     start=False, stop=True)
                    nc.scalar.copy(out=qtab[:, b - 1, :], in_=p2[:, :])

            # --- phase 2: main edge loop -----------------------------------
            with ExitStack() as mn:
                pz = mn.enter_context(
                    tc.tile_pool(name="pz", bufs=4, space="PSUM"))
                pyt = mn.enter_context(
                    tc.tile_pool(name="pyt", bufs=1, space="PSUM"))
                pqd = mn.enter_context(
                    tc.tile_pool(name="pqd", bufs=2, space="PSUM"))
                pseg = mn.enter_context(
                    tc.tile_pool(name="pseg", bufs=1, space="PSUM"))
                big = mn.enter_context(tc.tile_pool(name="big", bufs=2))
                wk = mn.enter_context(tc.tile_pool(name="wk", bufs=4))
                bo = mn.enter_context(tc.tile_pool(name="blockout", bufs=2))

                def load_block(b):
                    """DMA a block's inputs and build its node-major
                    one-hot. Called one block ahead so nothing gates the
                    block start."""
                    reT = big.tile([REF, eb], BF16, tag="reT")
                    q4 = eb // 4
                    for k in range(4):
                        nc.sync.dma_start(
                            out=reT[:, k * q4 : (k + 1) * q4],
                            in_=inp["reT"][:, b * eb + k * q4
                                           : b * eb + (k + 1) * q4])
                    hjT = big.tile([DIM, eb], BF16, tag="hjT")
                    q6 = eb // 8
                    for k in range(8):
                        nc.sync.dma_start(
                            out=hjT[:, k * q6 : (k + 1) * q6],
                            in_=inp["hjT"][:, b * eb + k * q6
                                           : b * eb + (k + 1) * q6])
                    dstbc = big.tile([128, eb], BF16, tag="dstbc")
                    for k in range(4):
                        nc.sync.dma_start(
                            out=dstbc[:, k * q4 : (k + 1) * q4],
                            in_=inp["dstbcT"][:, b * eb + k * q4
                                              : b * eb + (k + 1) * q4])
                    STw = big.tile([128, eb], BF16, tag="STw")
                    return reT, hjT, dstbc, STw

                def build_stw(blk, k):
                    """One quarter of the node-major one-hot for a block."""
                    _, _, dstbc, STw = blk
                    q4 = eb // 4
                    nc.vector.tensor_scalar(
                        out=STw[:, k * q4 : (k + 1) * q4],
                        in0=dstbc[:, k * q4 : (k + 1) * q4],
                        scalar1=wt["iotac"][:, 0:1], scalar2=None,
                        op0=OP.is_equal)

                cur = load_block(0)
                for k in range(4):
                    build_stw(cur, k)
                nxt = None
                for b in range(NBLK):
                    reT, hjT, _, STw = cur

                    ps_seg = pseg.tile([128, 144], F32, tag="seg")

                    # sigmoid chain: ew = 1/(1+exp(-(u+ew_b)))
                    e1 = bo.tile([128, tiles], F32, tag="e1")
                    nc.scalar.activation(
                        out=e1[:, :],
                        in_=ulog[:, b * tiles : (b + 1) * tiles],
                        func=AF.Exp, scale=-1.0, bias=ewbc[:, 0:1])
                    ewp = bo.tile([128, tiles], F32, tag="ewp")
                    nc.vector.tensor_scalar(out=ewp[:, :], in0=e1[:, :],
                                            scalar1=1.0, scalar2=None,
                                            op0=OP.add)
                    ewr = bo.tile([128, tiles], F32, tag="ewr")
                    nc.vector.reciprocal(out=ewr[:, :], in_=ewp[:, :])

                    # ---- software-pipelined quad loop (2-step skew) ----
                    state = [None] * (nq + 2)
                    stw_done = 0
                    for s in range(nq + 2):
                        if s < nq:
                            q0 = s * 4  # first tile of quad (block-local)
                            zA = pz.tile([128, 512], F32, tag="z")
                            zB = pz.tile([128, 512], F32, tag="z")
                            for i in range(4):
                                tl = q0 + i
                                zt = zA if i < 2 else zB
                                o = (i % 2) * 256
                                sl = slice(tl * 128, (tl + 1) * 128)
                                nc.tensor.matmul(
                                    zt[:, o : o + 256], reT[:, sl],
                                    wt["wre"][:, :], start=True, stop=False)
                                nc.tensor.matmul(
                                    zt[:, o : o + 256], hjT[:, sl],
                                    wt["whj"][:, :], start=False, stop=False)
                                nc.tensor.matmul(
                                    zt[:, o : o + 256], STw[:, sl],
                                    atab[:, b, :], start=False, stop=True)
                            qd = pqd.tile([128, 512], F32, tag="qd")
                            for i in range(4):
                                tl = q0 + i
                                nc.tensor.matmul(
                                    qd[:, i * 128 : (i + 1) * 128],
                                    STw[:, tl * 128 : (tl + 1) * 128],
                                    qtab[:, b, :], start=True, stop=True)
                            # edge-major one-hot S for this quad's tiles
                            S4 = wk.tile([128, 4, 128], BF16, tag="S4")
                            ti0 = b * tiles + q0
                            iota_b = _sap(wt["iota"], wt["iota"][:, :],
                                          wt["iota"][:, 0:1],
                                          [[0, 4], [1, 128]])
                            dst_b = _sap(dstloc, dstloc[:, :],
                                         dstloc[:, ti0 : ti0 + 1],
                                         [[1, 4], [0, 128]])
                            nc.vector.tensor_tensor(
                                out=S4[:, :, :], in0=iota_b, in1=dst_b,
                                op=OP.is_equal)
                            state[s] = [q0, zA, zB, qd, S4, None]

                        if s >= 2:
                            # ---- segment accumulate for quad s-2 ----
                            q0m2, _, _, _, S4m2, rhs_m2 = state[s - 2]
                            for i in range(4):
                                tl = q0m2 + i
                                nc.tensor.matmul(
                                    ps_seg[:, 0:144], S4m2[:, i, :],
                                    rhs_m2[:, i, :],
                                    start=(tl == 0), stop=(tl == tiles - 1))
                            state[s - 2] = None

                        if s == 2 and b + 1 < NBLK:
                            nxt = load_block(b + 1)
                        if 3 <= s and stw_done < 4 and b + 1 < NBLK:
                            build_stw(nxt, stw_done)
                            stw_done += 1

                        if s == 0 or s > nq:
                            continue
                        # ---- tail of quad s-1 ----
                        q0, zA, zB, qd, S4, _ = state[s - 1]

                        # uniform relu -> y (bf16); split across Scalar/Vector
                        y = wk.tile([128, 1024], BF16, tag="y")
                        nc.scalar.activation(out=y[:, 0:512], in_=zA[:, :],
                                             func=AF.Relu, scale=1.0, bias=0.0)
                        nc.vector.tensor_scalar(out=y[:, 512:1024],
                                                in0=zB[:, :], scalar1=0.0,
                                                scalar2=None, op0=OP.max)

                        # transposes of the 8 halves
                        ps_yt = pyt.tile([128, 512], F32, tag="yt")
                        for hh in range(8):
                            nc.tensor.transpose(
                                _bf(ps_yt[:, hh * 64 : (hh + 1) * 64]),
                                y[:, hh * 128 : (hh + 1) * 128],
                                wt["ident"][:, :])
                        # LN stats: mean==0 by construction -> only sum(z^2).
                        # Squares must be emitted before L2 (which overwrites
                        # zA/zB); Tile's WAR tracking orders the hardware.
                        z2 = wk.tile([128, 1024], BF16, tag="z2")
                        nc.scalar.activation(out=z2[:, 0:512], in_=zA[:, :],
                                             func=AF.Square, scale=1.0,
                                             bias=0.0)
                        nc.scalar.activation(out=z2[:, 512:1024],
                                             in_=zB[:, :], func=AF.Square,
                                             scale=1.0, bias=0.0)

                        ytS = wk.tile([128, 1024], BF16, tag="ytS")
                        nc.vector.tensor_copy(out=ytS[:, :],
                                              in_=_bf(ps_yt[:, :]))

                        # L2: K2 -> zA (reused), V2 -> zB (reused)
                        for i in range(4):
                            nc.tensor.matmul(
                                zA[:, i * 128 : (i + 1) * 128],
                                ytS[:, (2 * i) * 128 : (2 * i + 1) * 128],
                                wt["w2k"][:, :], start=True, stop=True)
                        for i in range(4):
                            nc.tensor.matmul(
                                zB[:, i * 128 : (i + 1) * 128],
                                ytS[:, (2 * i + 1) * 128 : (2 * i + 2) * 128],
                                wt["w2v"][:, :], start=True, stop=True)

                        qds = wk.tile([128, 512], F32, tag="qds")
                        nc.scalar.copy(out=qds[:, :], in_=qd[:, :])
                        mul = wk.tile([128, 512], F32, tag="mul")
                        nc.vector.tensor_tensor(out=mul[:, :], in0=zA[:, :],
                                                in1=qds[:, :], op=OP.mult)
                        s128 = wk.tile([128, 8], F32, tag="s128")
                        nc.vector.tensor_reduce(
                            out=s128[:, :],
                            in_=z2[:, :].rearrange("p (g f) -> p g f", g=8),
                            axis=mybir.AxisListType.X, op=OP.add)
                        lnv = wk.tile([128, 8], F32, tag="lnv")
                        nc.scalar.activation(out=lnv[:, :], in_=s128[:, :],
                                             func=AF.Ln, scale=1.0 / 128.0,
                                             bias=epsc[:, 0:1])
                        rstd = wk.tile([128, 8], F32, tag="rstd")
                        nc.scalar.activation(out=rstd[:, :], in_=lnv[:, :],
                                             func=AF.Exp, bias=0.0, scale=-0.5)
                        # per-head sum of 8: pairwise tree on GPSIMD
                        r1 = wk.tile([128, 256], F32, tag="r1")
                        nc.gpsimd.tensor_tensor(
                            out=r1[:, :],
                            in0=_sap(mul, mul[:, :], mul[:, 0:1], [[2, 256]]),
                            in1=_sap(mul, mul[:, :], mul[:, 1:2], [[2, 256]]),
                            op=OP.add)
                        r2 = wk.tile([128, 128], F32, tag="r2")
                        nc.gpsimd.tensor_tensor(
                            out=r2[:, :],
                            in0=_sap(r1, r1[:, :], r1[:, 0:1], [[2, 128]]),
                            in1=_sap(r1, r1[:, :], r1[:, 1:2], [[2, 128]]),
                            op=OP.add)
                        lred = wk.tile([128, 64], F32, tag="lred")
                        nc.gpsimd.tensor_tensor(
                            out=lred[:, :],
                            in0=_sap(r2, r2[:, :], r2[:, 0:1], [[2, 64]]),
                            in1=_sap(r2, r2[:, :], r2[:, 1:2], [[2, 64]]),
                            op=OP.add)
                        lsc = wk.tile([128, 64], F32, tag="lsc")
                        rk_ap = _sap(rstd, rstd[:, :], rstd[:, 0:1],
                                     [[2, 4], [0, 16]])
                        nc.gpsimd.tensor_tensor(out=lsc[:, :], in0=lred[:, :],
                                                in1=rk_ap, op=OP.mult)

                        # ex -> rhs[:, :, 128:144]
                        rhs = wk.tile([128, 4, 144], BF16, tag="rhs")
                        nc.scalar.activation(
                            out=rhs[:, :, 128:144],
                            in_=lsc[:, :].rearrange("p (t h) -> p t h", t=4),
                            func=AF.Exp, scale=RS8, bias=0.0)

                        # ewrv = (1/(1+e^-u)) * rstd_v  per tile
                        ewrv = wk.tile([128, 4], F32, tag="ewrv")
                        rv_ap = _sap(rstd, rstd[:, :], rstd[:, 1:2],
                                     [[2, 4]])
                        nc.gpsimd.tensor_tensor(
                            out=ewrv[:, :], in0=ewr[:, q0 : q0 + 4],
                            in1=rv_ap, op=OP.mult)
                        # exw = ex * ewrv
                        exw = wk.tile([128, 4, 16], F32, tag="exw")
                        ewrv_b = _sap(ewrv, ewrv[:, :], ewrv[:, 0:1],
                                      [[1, 4], [0, 16]])
                        nc.gpsimd.tensor_tensor(out=exw[:, :, :],
                                                in0=rhs[:, :, 128:144],
                                                in1=ewrv_b, op=OP.mult)

                        # vw = V2 * exw (per pair, 4-level APs)
                        for p2 in range(2):
                            o = p2 * 2
                            out_ap = _sap(rhs, rhs[:, :, :], rhs[:, o:, 0:1],
                                          [[144, 2], [8, 16], [1, 8]])
                            in0_ap = _sap(zB, zB[:, :], zB[:, o * 128:],
                                          [[128, 2], [8, 16], [1, 8]])
                            in1_ap = _sap(exw, exw[:, :, :], exw[:, o:, 0:1],
                                          [[16, 2], [1, 16], [0, 8]])
                            nc.vector.tensor_tensor(out=out_ap, in0=in0_ap,
                                                    in1=in1_ap, op=OP.mult)

                        state[s - 1][5] = rhs

                    # ---- block epilogue ----
                    dtmp = bo.tile([128, 16], F32, tag="dtmp")
                    nc.vector.tensor_scalar(
                        out=dtmp[:, :], in0=ps_seg[:, 128:144],
                        scalar1=DEN_EPS, scalar2=None, op0=OP.add)
                    dinv = bo.tile([128, 16], F32, tag="dinv")
                    nc.vector.reciprocal(out=dinv[:, :], in_=dtmp[:, :])
                    dinvb = _sap(dinv, dinv[:, :], dinv[:, 0:1],
                                 [[1, 16], [0, 8]])
                    aggs = bo.tile([128, 128], BF16, tag="aggs")
                    nc.vector.tensor_tensor(
                        out=aggs[:, :].rearrange("p (h d) -> p h d", h=16),
                        in0=ps_seg[:, 0:128].rearrange("p (h d) -> p h d",
                                                       h=16),
                        in1=dinvb, op=OP.mult)

                    ps_ep = pqd.tile([128, 512], F32, tag="qd")
                    nc.tensor.transpose(_bf(ps_ep[:, 256:320]), aggs[:, :],
                                        wt["ident"][:, :])
                    aT = bo.tile([128, 128], BF16, tag="aT")
                    nc.scalar.copy(out=aT[:, :], in_=_bf(ps_ep[:, 256:320]))
                    nc.tensor.matmul(ps_ep[:, 0:128], aT[:, :],
                                     wt["wno1a"][:, :], start=True, stop=False)
                    nc.tensor.matmul(ps_ep[:, 0:128],
                                     hTc[:, b * 128 : (b + 1) * 128],
                                     wt["wno1h"][:, :], start=False,
                                     stop=False)
                    nc.tensor.matmul(ps_ep[:, 0:128], ones1[:, :],
                                     wt["bno1"][:, :], start=False, stop=True)
                    rstd_o, nmr_o = _ln_chain(
                        nc, bo, ps_ep[:, 0:128].rearrange("p (o f) -> p o f",
                                                          o=1), 1, "o",
                        epsc[:, 0:1])
                    yno = bo.tile([128, 128], BF16, tag="yno")
                    nc.scalar.activation(out=yno[:, :], in_=ps_ep[:, 0:128],
                                         func=AF.Relu, scale=rstd_o[:, 0:1],
                                         bias=nmr_o[:, 0:1])
                    nc.tensor.transpose(_bf(ps_ep[:, 320:384]), yno[:, :],
                                        wt["ident"][:, :])
                    ynoT = bo.tile([128, 128], BF16, tag="ynoT")
                    nc.vector.tensor_copy(out=ynoT[:, :],
                                          in_=_bf(ps_ep[:, 320:384]))
                    nc.tensor.matmul(ps_ep[:, 128:256], ynoT[:, :],
                                     wt["wno2"][:, :], start=True, stop=False)
                    nc.tensor.matmul(ps_ep[:, 128:256], ones1[:, :],
                                     wt["bno2"][:, :], start=False, stop=True)
                    outt = bo.tile([128, 128], F32, tag="outt")
                    nc.vector.tensor_tensor(out=outt[:, :],
                                            in0=ps_ep[:, 128:256],
                                            in1=hrows[:, b, :], op=OP.add)
                    nc.sync.dma_start(
                        out=out_d[b * 128 : (b + 1) * 128, :], in_=outt[:, :])
                    if b + 1 < NBLK:
                        while stw_done < 4:
                            build_stw(nxt, stw_done)
                            stw_done += 1
                    cur = nxt

    _split_multiwait_drains(nc)
    return nc


# ---------------------------------------------------------------------------
# entry point
# ---------------------------------------------------------------------------

_CACHE = {}
LAST_RESULT = {}


def kernel(**inputs):
    _install_ntff_hook_shim()
    wts = _prep_weights_v2(inputs)
    if wts is None:
        from kernel_v1_backup import kernel as kernel_v1  # pragma: no cover

        return kernel_v1(**inputs)

    per_core, eb = _prep_inputs_v2(inputs)
    key = ("v2", eb, round(wts["ewb"], 9))
    if key not in _CACHE:
        _CACHE[key] = build_program_v2(eb, wts["ewb"])
    nc = _CACHE[key]

    wt_arrays = {}
    for k, (shp, dt) in WT_SHAPES_V2.items():
        a = np.ascontiguousarray(wts[k])
        wt_arrays[k] = a.astype(NPBF) if dt == BF16 else a.astype(np.float32)
    in_maps = []
    for c in range(NCORES):
        m = dict(per_core[c])
        m.update(wt_arrays)
        in_maps.append(m)

    trace = bool(int(os.environ.get("KERNEL_TRACE", "0")))
    res = run_bass_kernel_spmd(nc, in_maps, list(range(NCORES)), trace=trace)
    LAST_RESULT["res"] = res

    out = np.concatenate([res.results[c]["out"] for c in range(NCORES)],
                         axis=0)
    return np.ascontiguousarray(out[:N]).astype(np.float32)


# revision 55
# speedup vs baseline: 1.1829x; 1.1829x over previous
"""Trainium2 Bass kernel for nn_BaseX2HAttLayer (GNN edge-attention layer).

Strategy (v2)
-------------
Host: stable-sort edges by dst. Pad nodes to 10240 = 8 cores x 10 blocks x
128. Each core owns 1280 contiguous dst nodes and their edges (softmax
segments never cross cores -> no collectives). Edges per 128-node block are
padded to eb = tiles*128 with tiles a multiple of 4; pad edges get dst
slot -1 so their one-hot row is all-zero and they contribute nothing.

Key algebraic moves (all pure weight reparameterizations done on host):
  * W1 columns of the edge MLPs (hk, hv) are centered per 128-wide hidden
    half, so the LayerNorm mean is identically zero -> the per-tile
    scale/bias activation collapses to one uniform wide ReLU per PSUM bank.
  * relu(r*x) = r*relu(x) for r>0: the LN inv-std is applied AFTER the
    second linear layer - on the k side it multiplies the logits (one wide
    tensor op), on the v side it rides along with the e_w sigmoid factor.
  * b2 biases of hk/hv are zero for this problem (asserted; falls back to
    the v1 program otherwise).

Device per core: per 128-node block, edges are processed in quads (4 tiles
of 128 edges). PSUM: 4 rotating z-banks (pairs of tiles; reused for the
second-layer outputs), 2 q-gather banks, 1 transpose bank, 1 segment
accumulator (+ e_w logit slack). The tensor engine is kept continuously
busy (HAM warm state) by emitting quad q's L1 matmuls ahead of quad q-1's
tail. bn_stats is grouped per bank; softmax/e_w/logit work is spread
across Vector / Scalar / GPSIMD.
"""

import os
import sys

sys.path.insert(0, "/opt/trn_rl_repo")

import ml_dtypes
import numpy as np

import concourse.bass as bass
import concourse.mybir as mybir
from concourse.bass_utils import run_bass_kernel_spmd
from concourse.tile import TileContext

F32 = mybir.dt.float32
BF16 = mybir.dt.bfloat16
AF = mybir.ActivationFunctionType
OP = mybir.AluOpType
NPBF = ml_dtypes.bfloat16

N, E = 10000, 320000
DIM = 128
NH, HD = 16, 8
EFD, RFD = 4, 64
REF = EFD + RFD  # 68
NCORES = 8
NPAD = 10240
NPC = NPAD // NCORES  # 1280
NBLK = NPC // 128  # 10
LN_EPS = 1e-5
DEN_EPS = 1e-16
RS8 = float(1.0 / np.sqrt(HD))


def _bf(ap):
    return ap.bitcast(BF16)


def _sap(tile, base_ap, offset_ap, dims):
    """Manual AP on tile with explicit [stride, count] free dims."""
    return bass.AP(tensor=tile.tensor, offset=offset_ap.offset,
                   ap=[base_ap.ap[0]] + dims)


# ---------------------------------------------------------------------------
# compile-path workarounds (this image)
# ---------------------------------------------------------------------------


def _split_multiwait_drains(nc):
    """This walrus build allows few sync-waits per instruction. Hoist excess
    waits onto single-wait Drains inserted just before, on the same engine."""
    ctr = [0]
    for fn in nc.m.functions:
        for bb in fn.blocks:
            out = []
            for ins in bb.instructions:
                si = ins.sync_info
                limit = 1
                if si is not None and len(si.on_wait) > limit:
                    waits = list(si.on_wait)
                    for w in waits[:-limit]:
                        d = mybir.InstDrain(
                            name=f"I-splitw-{ctr[0]}", ins=[], outs=[]
                        )
                        ctr[0] += 1
                        d.engine = ins.engine
                        d.sync_info = mybir.SyncInfo(on_wait=[w], on_update=[])
                        nc.register_instruction(d, overwrite=True)
                        out.append(d)
                    ins.sync_info = mybir.SyncInfo(
                        on_wait=waits[-limit:], on_update=list(si.on_update)
                    )
                out.append(ins)
            bb.instructions[:] = out


def _install_ntff_hook_shim():
    import types

    if "antenv.axon_hooks" in sys.modules:
        return
    import antenv

    mod = types.ModuleType("antenv.axon_hooks")
    state = {"hook": None, "init": False}

    def set_axon_ntff_profile_hook(hook):
        state["hook"] = hook
        state["init"] = True

    def get_axon_ntff_profile_hook():
        if not state["init"]:
            try:
                from trn_agent_boot.trn_boot import _ntff_profile_via_ctypes

                state["hook"] = _ntff_profile_via_ctypes(
                    "/opt/axon/libaxon_pjrt.so"
                )
            except Exception:
                state["hook"] = None
            state["init"] = True
        return state["hook"]

    mod.set_axon_ntff_profile_hook = set_axon_ntff_profile_hook
    mod.get_axon_ntff_profile_hook = get_axon_ntff_profile_hook
    sys.modules["antenv.axon_hooks"] = mod
    antenv.axon_hooks = mod


# ---------------------------------------------------------------------------
# host-side prep (v2)
# ---------------------------------------------------------------------------


def _prep_inputs_v2(inputs):
    h = np.asarray(inputs["h"], np.float32)
    r_feat = np.asarray(inputs["r_feat"], np.float32)
    edge_feat = np.asarray(inputs["edge_feat"], np.float32)
    ei = np.asarray(inputs["edge_index"])
    src, dst = ei[0].astype(np.int64), ei[1].astype(np.int64)

    order = np.argsort(dst, kind="stable")
    src_s, dst_s = src[order], dst[order]
    ref_s = np.concatenate([edge_feat[order], r_feat[order]], axis=1)  # [E,68]
    ew_W = np.asarray(inputs["ew_W"], np.float32)
    u_s = (r_feat @ ew_W)[:, 0][order]  # e_w gate logit (pre-bias)

    nblk_tot = NPAD // 128
    starts = np.searchsorted(dst_s, np.arange(nblk_tot) * 128)
    ends = np.searchsorted(dst_s, (np.arange(nblk_tot) + 1) * 128)
    cnts = ends - starts
    tiles = int((cnts.max() + 127) // 128)
    tiles = ((tiles + 3) // 4) * 4
    eb = tiles * 128

    hpad = np.zeros((NPAD, DIM), np.float32)
    hpad[:N] = h

    per_core = []
    for c in range(NCORES):
        reT = np.zeros((REF, NBLK * eb), np.float32)
        hjT = np.zeros((DIM, NBLK * eb), np.float32)
        dstloc = np.full((128, NBLK * tiles), -1.0, np.float32)
        ulog = np.zeros((128, NBLK * tiles), np.float32)
        dstT = np.full((NBLK * eb,), -1.0, np.float32)
        for b in range(NBLK):
            g = c * NBLK + b
            s0, cnt = starts[g], cnts[g]
            sl = slice(s0, s0 + cnt)
            reT[:, b * eb : b * eb + cnt] = ref_s[sl].T
            hjT[:, b * eb : b * eb + cnt] = hpad[src_s[sl]].T
            dl = np.full(eb, -1.0, np.float32)
            dl[:cnt] = (dst_s[sl] - g * 128).astype(np.float32)
            dstloc[:, b * tiles : (b + 1) * tiles] = dl.reshape(tiles, 128).T
            dstT[b * eb : b * eb + cnt] = dl[:cnt]
            ul = np.zeros(eb, np.float32)
            ul[:cnt] = u_s[sl]
            ulog[:, b * tiles : (b + 1) * tiles] = ul.reshape(tiles, 128).T
        dstbcT = np.broadcast_to(
            dstT.astype(NPBF)[None, :], (128, NBLK * eb)).copy()
        hrows = np.zeros((128, NBLK, DIM), np.float32)
        blkn = hpad[c * NPC : (c + 1) * NPC].reshape(NBLK, 128, DIM)
        hrows[:, :, :] = blkn.transpose(1, 0, 2)
        hTc = np.ascontiguousarray(hpad[c * NPC : (c + 1) * NPC].T).astype(NPBF)
        per_core.append(
            {"reT": reT.astype(NPBF), "hjT": hjT.astype(NPBF),
             "dstloc": dstloc, "ulog": ulog, "dstbcT": dstbcT,
             "hrows": hrows, "hTc": hTc}
        )
    return per_core, eb


def _center_cols(W):
    """Subtract per-row mean over output columns (makes LN mean exactly 0)."""
    return W - W.mean(axis=1, keepdims=True)


def _prep_weights_v2(inputs):
    g = {k: np.asarray(v, np.float32) for k, v in inputs.items()
         if k != "edge_index"}
    for nm in ("hk", "hv", "hq", "no"):
        assert np.allclose(g[f"{nm}_g1"], 1.0) and np.allclose(
            g[f"{nm}_be1"], 0.0
        ), "LN affine folding requires g1=1, be1=0"

    ok = (not np.any(g["hk_b2"] != 0.0)) and (not np.any(g["hv_b2"] != 0.0))
    if not ok:
        return None  # caller falls back to v1 program

    w = {}
    kW1 = _center_cols(g["hk_W1"])  # [324,128]
    vW1 = _center_cols(g["hv_W1"])
    w["wre"] = np.concatenate([kW1[:REF], vW1[:REF]], axis=1)  # [68,256]
    w["whi"] = np.concatenate(
        [kW1[REF : REF + DIM], vW1[REF : REF + DIM]], axis=1)  # [128,256]
    w["whj"] = np.concatenate([kW1[REF + DIM :], vW1[REF + DIM :]], 1)
    b1k = g["hk_b1"] - g["hk_b1"].mean()
    b1v = g["hv_b1"] - g["hv_b1"].mean()
    w["b1c"] = np.concatenate([b1k, b1v])[None, :]  # [1,256]
    w["w2k"] = g["hk_W2"]
    w["w2v"] = g["hv_W2"]
    w["ewWn"] = np.zeros((REF, 1), np.float32)
    w["ewWn"][EFD:, 0] = -g["ew_W"][:, 0]
    w["ewb"] = float(g["ew_b"][0])
    # q-MLP (computed with full LN on device in phase 1)
    w["wq1"] = g["hq_W1"]
    w["bq1"] = g["hq_b1"][None]
    w["wq2"] = g["hq_W2"]
    w["bq2"] = g["hq_b2"][None]
    # out-MLP
    w["wno1a"] = g["no_W1"][:DIM]
    w["wno1h"] = g["no_W1"][DIM:]
    w["bno1"] = g["no_b1"][None]
    w["wno2"] = g["no_W2"]
    w["bno2"] = g["no_b2"][None]
    w["iota"] = np.broadcast_to(
        np.arange(128, dtype=np.float32), (128, 128)).copy()  # row-arange
    w["iotac"] = np.arange(128, dtype=np.float32)[:, None]  # [128,1]
    w["ident"] = np.eye(128, dtype=np.float32)
    return w


WT_SHAPES_V2 = {
    "wre": ((REF, 256), BF16), "whi": ((DIM, 256), BF16),
    "whj": ((DIM, 256), BF16), "b1c": ((1, 256), BF16),
    "w2k": ((DIM, DIM), BF16), "w2v": ((DIM, DIM), BF16),
    "ewWn": ((REF, 1), BF16),
    "wq1": ((DIM, DIM), BF16), "bq1": ((1, DIM), BF16),
    "wq2": ((DIM, DIM), BF16), "bq2": ((1, DIM), BF16),
    "wno1a": ((DIM, DIM), BF16), "wno1h": ((DIM, DIM), BF16),
    "bno1": ((1, DIM), BF16), "wno2": ((DIM, DIM), BF16),
    "bno2": ((1, DIM), BF16),
    "iota": ((128, 128), BF16), "iotac": ((128, 1), F32),
    "ident": ((128, 128), BF16),
}


# ---------------------------------------------------------------------------
# device program (v2)
# ---------------------------------------------------------------------------


def _ln_chain(nc, wk, psum_src, nhalves, name, eps_ap):
    stats = wk.tile([128, nhalves, 6], F32, tag=f"st{name}")
    mv = wk.tile([128, nhalves, 2], F32, tag=f"mv{name}")
    for hh in range(nhalves):
        nc.vector.bn_stats(out=stats[:, hh, :], in_=psum_src[:, hh, :])
        nc.vector.bn_aggr(out=mv[:, hh, :], in_=stats[:, hh, :])
    lnv = wk.tile([128, nhalves], F32, tag=f"lnv{name}")
    nc.scalar.activation(out=lnv[:, :], in_=mv[:, :, 1], func=AF.Ln,
                         bias=eps_ap, scale=1.0)
    rstd = wk.tile([128, nhalves], F32, tag=f"rstd{name}")
    nc.scalar.activation(out=rstd[:, :], in_=lnv[:, :], func=AF.Exp,
                         bias=0.0, scale=-0.5)
    negmu = wk.tile([128, nhalves], F32, tag=f"ngm{name}")
    nc.vector.tensor_scalar(out=negmu[:, :], in0=mv[:, :, 0], scalar1=-1.0,
                            scalar2=None, op0=OP.mult)
    nmr = wk.tile([128, nhalves], F32, tag=f"nmr{name}")
    nc.vector.tensor_tensor(out=nmr[:, :], in0=negmu[:, :], in1=rstd[:, :],
                            op=OP.mult)
    return rstd, nmr


def build_program_v2(eb, ewb):
    tiles = eb // 128
    nq = tiles // 4
    nc = bass.Bass()

    inp = {}
    inp["reT"] = nc.declare_dram_parameter("reT", [REF, NBLK * eb], BF16,
                                           isOutput=False)
    inp["hjT"] = nc.declare_dram_parameter("hjT", [DIM, NBLK * eb], BF16,
                                           isOutput=False)
    inp["dstloc"] = nc.declare_dram_parameter("dstloc", [128, NBLK * tiles],
                                              F32, isOutput=False)
    inp["ulog"] = nc.declare_dram_parameter("ulog", [128, NBLK * tiles],
                                            F32, isOutput=False)
    inp["dstbcT"] = nc.declare_dram_parameter("dstbcT", [128, NBLK * eb],
                                              BF16, isOutput=False)
    inp["hTc"] = nc.declare_dram_parameter("hTc", [128, NBLK * 128], BF16,
                                           isOutput=False)
    inp["hrows"] = nc.declare_dram_parameter("hrows", [128, NBLK, DIM], F32,
                                             isOutput=False)
    for k, (shp, dt) in WT_SHAPES_V2.items():
        inp[k] = nc.declare_dram_parameter(k, list(shp), dt, isOutput=False)
    out_d = nc.declare_dram_parameter("out", [NPC, DIM], F32, isOutput=True)

    with TileContext(nc, num_cores=NCORES) as tc:
        from contextlib import ExitStack

        with ExitStack() as ctx:
            sg = ctx.enter_context(tc.tile_pool(name="singles", bufs=1))

            # phase-1-critical weights first so the PE can start promptly
            _ord = ["whi", "b1c", "wq1", "bq1", "ident", "wq2", "bq2"]
            _ord += [k for k in WT_SHAPES_V2 if k not in _ord]
            wt = {}
            for k in _ord:
                shp, dt = WT_SHAPES_V2[k]
                wt[k] = sg.tile(list(shp), dt, name=f"wt_{k}", tag=f"wt_{k}")
                nc.sync.dma_start(out=wt[k][:, :], in_=inp[k][:, :])
            ones1 = sg.tile([1, 128], BF16)
            nc.vector.memset(ones1, 1.0)
            epsc = sg.tile([128, 1], F32)
            nc.vector.memset(epsc, LN_EPS)
            ewbc = sg.tile([128, 1], F32)
            nc.vector.memset(ewbc, -ewb)
            hTc = sg.tile([128, NBLK * 128], BF16)
            for k in range(5):
                nc.sync.dma_start(
                    out=hTc[:, k * 256 : (k + 1) * 256],
                    in_=inp["hTc"][:, k * 256 : (k + 1) * 256])
            dstloc = sg.tile([128, NBLK * tiles], F32)
            nc.sync.dma_start(out=dstloc[:, :], in_=inp["dstloc"][:, :])
            ulog = sg.tile([128, NBLK * tiles], F32)
            nc.sync.dma_start(out=ulog[:, :], in_=inp["ulog"][:, :])
            hrows = sg.tile([128, NBLK, DIM], F32)
            nc.sync.dma_start(out=hrows[:, :, :], in_=inp["hrows"][:, :, :])
            atab = sg.tile([128, NBLK, 256], BF16)
            qtab = sg.tile([128, NBLK, 128], BF16)

            # --- phase 1: atab (centered hi-part of L1) and q table --------
            with ExitStack() as pre:
                pp = pre.enter_context(
                    tc.tile_pool(name="prepsum", bufs=2, space="PSUM"))
                pw = pre.enter_context(tc.tile_pool(name="prework", bufs=4))

                pst = [None] * NBLK
                for b in range(NBLK + 1):
                    if b < NBLK:
                        hTb = hTc[:, b * 128 : (b + 1) * 128]
                        ps = pp.tile([128, 256], F32, tag="Ap")
                        nc.tensor.matmul(ps[:, :], hTb, wt["whi"][:, :],
                                         start=True, stop=False)
                        nc.tensor.matmul(ps[:, :], ones1[:, :],
                                         wt["b1c"][:, :],
                                         start=False, stop=True)
                        p1 = pp.tile([128, 128], F32, tag="q1")
                        nc.tensor.matmul(p1[:, :], hTb, wt["wq1"][:, :],
                                         start=True, stop=False)
                        nc.tensor.matmul(p1[:, :], ones1[:, :],
                                         wt["bq1"][:, :],
                                         start=False, stop=True)
                        pst[b] = (ps, p1)
                    if b == 0:
                        continue
                    ps, p1 = pst[b - 1]
                    pst[b - 1] = None
                    nc.scalar.copy(out=atab[:, b - 1, :], in_=ps[:, :])
                    rstd, nmr = _ln_chain(
                        nc, pw, p1[:, :].rearrange("p (o f) -> p o f", o=1),
                        1, "q", epsc[:, 0:1])
                    yq = pw.tile([128, 128], BF16, tag="yq")
                    nc.scalar.activation(out=yq[:, :], in_=p1[:, :],
                                         func=AF.Relu, scale=rstd[:, 0:1],
                                         bias=nmr[:, 0:1])
                    pt = pp.tile([128, 64], F32, tag="qT")
                    nc.tensor.transpose(_bf(pt[:, :]), yq[:, :],
                                        wt["ident"][:, :])
                    yqT = pw.tile([128, 128], BF16, tag="yqT")
                    nc.vector.tensor_copy(out=yqT[:, :], in_=_bf(pt[:, :]))
                    p2 = pp.tile([128, 128], F32, tag="q2")
                    nc.tensor.matmul(p2[:, :], yqT[:, :], wt["wq2"][:, :],
                                     start=True, stop=False)
                    nc.tensor.matmul(p2[:, :], ones1[:, :], wt["bq2"][:, :],
                                     start=False, stop=True)
                    nc.scalar.copy(out=qtab[:, b - 1, :], in_=p2[:, :])

            # --- phase 2: main edge loop -----------------------------------
            with ExitStack() as mn:
                pz = mn.enter_context(
                    tc.tile_pool(name="pz", bufs=4, space="PSUM"))
                pyt = mn.enter_context(
                    tc.tile_pool(name="pyt", bufs=1, space="PSUM"))
                pqd = mn.enter_context(
                    tc.tile_pool(name="pqd", bufs=2, space="PSUM"))
                pseg = mn.enter_context(
                    tc.tile_pool(name="pseg", bufs=1, space="PSUM"))
                big = mn.enter_context(tc.tile_pool(name="big", bufs=2))
                wk = mn.enter_context(tc.tile_pool(name="wk", bufs=3))
                bo = mn.enter_context(tc.tile_pool(name="blockout", bufs=2))

                def load_block(b):
                    """DMA a block's inputs and build its node-major
                    one-hot. Called one block ahead so nothing gates the
                    block start."""
                    reT = big.tile([REF, eb], BF16, tag="reT")
                    q4 = eb // 4
                    for k in range(4):
                        nc.sync.dma_start(
                            out=reT[:, k * q4 : (k + 1) * q4],
                            in_=inp["reT"][:, b * eb + k * q4
                                           : b * eb + (k + 1) * q4])
                    hjT = big.tile([DIM, eb], BF16, tag="hjT")
                    q6 = eb // 8
                    for k in range(8):
                        nc.sync.dma_start(
                            out=hjT[:, k * q6 : (k + 1) * q6],
                            in_=inp["hjT"][:, b * eb + k * q6
                                           : b * eb + (k + 1) * q6])
                    dstbc = big.tile([128, eb], BF16, tag="dstbc")
                    for k in range(4):
                        nc.sync.dma_start(
                            out=dstbc[:, k * q4 : (k + 1) * q4],
                            in_=inp["dstbcT"][:, b * eb + k * q4
                                              : b * eb + (k + 1) * q4])
                    STw = big.tile([128, eb], BF16, tag="STw")
                    return reT, hjT, dstbc, STw

                def build_stw(blk, k):
                    """One quarter of the node-major one-hot for a block."""
                    _, _, dstbc, STw = blk
                    q4 = eb // 4
                    nc.vector.tensor_scalar(
                        out=STw[:, k * q4 : (k + 1) * q4],
                        in0=dstbc[:, k * q4 : (k + 1) * q4],
                        scalar1=wt["iotac"][:, 0:1], scalar2=None,
                        op0=OP.is_equal)

                cur = load_block(0)
                for k in range(4):
                    build_stw(cur, k)
                nxt = None
                for b in range(NBLK):
                    reT, hjT, _, STw = cur

                    ps_seg = pseg.tile([128, 144], F32, tag="seg")

                    # sigmoid chain: ew = 1/(1+exp(-(u+ew_b)))
                    e1 = bo.tile([128, tiles], F32, tag="e1")
                    nc.scalar.activation(
                        out=e1[:, :],
                        in_=ulog[:, b * tiles : (b + 1) * tiles],
                        func=AF.Exp, scale=-1.0, bias=ewbc[:, 0:1])
                    ewp = bo.tile([128, tiles], F32, tag="ewp")
                    nc.vector.tensor_scalar(out=ewp[:, :], in0=e1[:, :],
                                            scalar1=1.0, scalar2=None,
                                            op0=OP.add)
                    ewr = bo.tile([128, tiles], F32, tag="ewr")
                    nc.vector.reciprocal(out=ewr[:, :], in_=ewp[:, :])

                    # ---- software-pipelined quad loop (2-step skew) ----
                    state = [None] * (nq + 3)
                    stw_done = 0
                    for s in range(nq + 3):
                        if s < nq:
                            q0 = s * 4  # first tile of quad (block-local)
                            zA = pz.tile([128, 512], F32, tag="z")
                            zB = pz.tile([128, 512], F32, tag="z")
                            for i in range(4):
                                tl = q0 + i
                                zt = zA if i < 2 else zB
                                o = (i % 2) * 256
                                sl = slice(tl * 128, (tl + 1) * 128)
                                nc.tensor.matmul(
                                    zt[:, o : o + 256], reT[:, sl],
                                    wt["wre"][:, :], start=True, stop=False)
                                nc.tensor.matmul(
                                    zt[:, o : o + 256], hjT[:, sl],
                                    wt["whj"][:, :], start=False, stop=False)
                                nc.tensor.matmul(
                                    zt[:, o : o + 256], STw[:, sl],
                                    atab[:, b, :], start=False, stop=True)
                            qd = pqd.tile([128, 512], F32, tag="qd")
                            for i in range(4):
                                tl = q0 + i
                                nc.tensor.matmul(
                                    qd[:, i * 128 : (i + 1) * 128],
                                    STw[:, tl * 128 : (tl + 1) * 128],
                                    qtab[:, b, :], start=True, stop=True)
                            # edge-major one-hot S for this quad's tiles
                            S4 = wk.tile([128, 4, 128], BF16, tag="S4")
                            ti0 = b * tiles + q0
                            iota_b = _sap(wt["iota"], wt["iota"][:, :],
                                          wt["iota"][:, 0:1],
                                          [[0, 4], [1, 128]])
                            dst_b = _sap(dstloc, dstloc[:, :],
                                         dstloc[:, ti0 : ti0 + 1],
                                         [[1, 4], [0, 128]])
                            nc.vector.tensor_tensor(
                                out=S4[:, :, :], in0=iota_b, in1=dst_b,
                                op=OP.is_equal)
                            state[s] = [q0, zA, zB, qd, S4, None]

                        if s >= 3:
                            # ---- segment accumulate for quad s-3 ----
                            q0m2, _, _, _, S4m2, rhs_m2 = state[s - 3]
                            for i in range(4):
                                tl = q0m2 + i
                                nc.tensor.matmul(
                                    ps_seg[:, 0:144], S4m2[:, i, :],
                                    rhs_m2[:, i, :],
                                    start=(tl == 0), stop=(tl == tiles - 1))
                            state[s - 3] = None

                        if s == 2 and b + 1 < NBLK:
                            nxt = load_block(b + 1)
                        if 3 <= s and stw_done < 4 and b + 1 < NBLK:
                            build_stw(nxt, stw_done)
                            stw_done += 1

                        if s == 0 or s > nq:
                            continue
                        # ---- tail of quad s-1 ----
                        q0, zA, zB, qd, S4, _ = state[s - 1]

                        # uniform relu -> y (bf16); split across Scalar/Vector
                        y = wk.tile([128, 1024], BF16, tag="y")
                        nc.scalar.activation(out=y[:, 0:512], in_=zA[:, :],
                                             func=AF.Relu, scale=1.0, bias=0.0)
                        nc.vector.tensor_scalar(out=y[:, 512:1024],
                                                in0=zB[:, :], scalar1=0.0,
                                                scalar2=None, op0=OP.max)

                        # transposes of the 8 halves
                        ps_yt = pyt.tile([128, 512], F32, tag="yt")
                        for hh in range(8):
                            nc.tensor.transpose(
                                _bf(ps_yt[:, hh * 64 : (hh + 1) * 64]),
                                y[:, hh * 128 : (hh + 1) * 128],
                                wt["ident"][:, :])
                        # LN stats: mean==0 by construction -> only sum(z^2).
                        # Squares must be emitted before L2 (which overwrites
                        # zA/zB); Tile's WAR tracking orders the hardware.
                        z2 = wk.tile([128, 1024], BF16, tag="z2")
                        nc.scalar.activation(out=z2[:, 0:512], in_=zA[:, :],
                                             func=AF.Square, scale=1.0,
                                             bias=0.0)
                        nc.scalar.activation(out=z2[:, 512:1024],
                                             in_=zB[:, :], func=AF.Square,
                                             scale=1.0, bias=0.0)

                        ytS = wk.tile([128, 1024], BF16, tag="ytS")
                        nc.vector.tensor_copy(out=ytS[:, :],
                                              in_=_bf(ps_yt[:, :]))

                        # L2: K2 -> zA (reused), V2 -> zB (reused)
                        for i in range(4):
                            nc.tensor.matmul(
                                zA[:, i * 128 : (i + 1) * 128],
                                ytS[:, (2 * i) * 128 : (2 * i + 1) * 128],
                                wt["w2k"][:, :], start=True, stop=True)
                        for i in range(4):
                            nc.tensor.matmul(
                                zB[:, i * 128 : (i + 1) * 128],
                                ytS[:, (2 * i + 1) * 128 : (2 * i + 2) * 128],
                                wt["w2v"][:, :], start=True, stop=True)

                        qds = wk.tile([128, 512], F32, tag="qds")
                        nc.scalar.copy(out=qds[:, :], in_=qd[:, :])
                        mul = wk.tile([128, 512], F32, tag="mul")
                        nc.vector.tensor_tensor(out=mul[:, :], in0=zA[:, :],
                                                in1=qds[:, :], op=OP.mult)
                        s128 = wk.tile([128, 8], F32, tag="s128")
                        nc.vector.tensor_reduce(
                            out=s128[:, :],
                            in_=z2[:, :].rearrange("p (g f) -> p g f", g=8),
                            axis=mybir.AxisListType.X, op=OP.add)
                        lnv = wk.tile([128, 8], F32, tag="lnv")
                        nc.scalar.activation(out=lnv[:, :], in_=s128[:, :],
                                             func=AF.Ln, scale=1.0 / 128.0,
                                             bias=epsc[:, 0:1])
                        rstd = wk.tile([128, 8], F32, tag="rstd")
                        nc.scalar.activation(out=rstd[:, :], in_=lnv[:, :],
                                             func=AF.Exp, bias=0.0, scale=-0.5)
                        # per-head sum of 8: pairwise tree on GPSIMD
                        r1 = wk.tile([128, 256], F32, tag="r1")
                        nc.gpsimd.tensor_tensor(
                            out=r1[:, :],
                            in0=_sap(mul, mul[:, :], mul[:, 0:1], [[2, 256]]),
                            in1=_sap(mul, mul[:, :], mul[:, 1:2], [[2, 256]]),
                            op=OP.add)
                        r2 = wk.tile([128, 128], F32, tag="r2")
                        nc.gpsimd.tensor_tensor(
                            out=r2[:, :],
                            in0=_sap(r1, r1[:, :], r1[:, 0:1], [[2, 128]]),
                            in1=_sap(r1, r1[:, :], r1[:, 1:2], [[2, 128]]),
                            op=OP.add)
                        lred = wk.tile([128, 64], F32, tag="lred")
                        nc.gpsimd.tensor_tensor(
                            out=lred[:, :],
                            in0=_sap(r2, r2[:, :], r2[:, 0:1], [[2, 64]]),
                            in1=_sap(r2, r2[:, :], r2[:, 1:2], [[2, 64]]),
                            op=OP.add)
                        lsc = wk.tile([128, 64], F32, tag="lsc")
                        rk_ap = _sap(rstd, rstd[:, :], rstd[:, 0:1],
                                     [[2, 4], [0, 16]])
                        nc.gpsimd.tensor_tensor(out=lsc[:, :], in0=lred[:, :],
                                                in1=rk_ap, op=OP.mult)

                        # ex -> rhs[:, :, 128:144]
                        rhs = wk.tile([128, 4, 144], BF16, tag="rhs")
                        nc.scalar.activation(
                            out=rhs[:, :, 128:144],
                            in_=lsc[:, :].rearrange("p (t h) -> p t h", t=4),
                            func=AF.Exp, scale=RS8, bias=0.0)

                        # ewrv = (1/(1+e^-u)) * rstd_v  per tile
                        ewrv = wk.tile([128, 4], F32, tag="ewrv")
                        rv_ap = _sap(rstd, rstd[:, :], rstd[:, 1:2],
                                     [[2, 4]])
                        nc.gpsimd.tensor_tensor(
                            out=ewrv[:, :], in0=ewr[:, q0 : q0 + 4],
                            in1=rv_ap, op=OP.mult)
                        # exw = ex * ewrv
                        exw = wk.tile([128, 4, 16], F32, tag="exw")
                        ewrv_b = _sap(ewrv, ewrv[:, :], ewrv[:, 0:1],
                                      [[1, 4], [0, 16]])
                        nc.gpsimd.tensor_tensor(out=exw[:, :, :],
                                                in0=rhs[:, :, 128:144],
                                                in1=ewrv_b, op=OP.mult)

                        # vw = V2 * exw (per pair, 4-level APs)
                        for p2 in range(2):
                            o = p2 * 2
                            out_ap = _sap(rhs, rhs[:, :, :], rhs[:, o:, 0:1],
                                          [[144, 2], [8, 16], [1, 8]])
                            in0_ap = _sap(zB, zB[:, :], zB[:, o * 128:],
                                          [[128, 2], [8, 16], [1, 8]])
                            in1_ap = _sap(exw, exw[:, :, :], exw[:, o:, 0:1],
                                          [[16, 2], [1, 16], [0, 8]])
                            nc.vector.tensor_tensor(out=out_ap, in0=in0_ap,
                                                    in1=in1_ap, op=OP.mult)

                        state[s - 1][5] = rhs

                    # ---- block epilogue ----
                    dtmp = bo.tile([128, 16], F32, tag="dtmp")
                    nc.vector.tensor_scalar(
                        out=dtmp[:, :], in0=ps_seg[:, 128:144],
                        scalar1=DEN_EPS, scalar2=None, op0=OP.add)
                    dinv = bo.tile([128, 16], F32, tag="dinv")
                    nc.vector.reciprocal(out=dinv[:, :], in_=dtmp[:, :])
                    dinvb = _sap(dinv, dinv[:, :], dinv[:, 0:1],
                                 [[1, 16], [0, 8]])
                    aggs = bo.tile([128, 128], BF16, tag="aggs")
                    nc.vector.tensor_tensor(
                        out=aggs[:, :].rearrange("p (h d) -> p h d", h=16),
                        in0=ps_seg[:, 0:128].rearrange("p (h d) -> p h d",
                                                       h=16),
                        in1=dinvb, op=OP.mult)

                    ps_ep = pqd.tile([128, 512], F32, tag="qd")
                    nc.tensor.transpose(_bf(ps_ep[:, 256:320]), aggs[:, :],
                                        wt["ident"][:, :])
                    aT = bo.tile([128, 128], BF16, tag="aT")
                    nc.scalar.copy(out=aT[:, :], in_=_bf(ps_ep[:, 256:320]))
                    nc.tensor.matmul(ps_ep[:, 0:128], aT[:, :],
                                     wt["wno1a"][:, :], start=True, stop=False)
                    nc.tensor.matmul(ps_ep[:, 0:128],
                                     hTc[:, b * 128 : (b + 1) * 128],
                                     wt["wno1h"][:, :], start=False,
                                     stop=False)
                    nc.tensor.matmul(ps_ep[:, 0:128], ones1[:, :],
                                     wt["bno1"][:, :], start=False, stop=True)
                    rstd_o, nmr_o = _ln_chain(
                        nc, bo, ps_ep[:, 0:128].rearrange("p (o f) -> p o f",
                                                          o=1), 1, "o",
                        epsc[:, 0:1])
                    yno = bo.tile([128, 128], BF16, tag="yno")
                    nc.scalar.activation(out=yno[:, :], in_=ps_ep[:, 0:128],
                                         func=AF.Relu, scale=rstd_o[:, 0:1],
                                         bias=nmr_o[:, 0:1])
                    nc.tensor.transpose(_bf(ps_ep[:, 320:384]), yno[:, :],
                                        wt["ident"][:, :])
                    ynoT = bo.tile([128, 128], BF16, tag="ynoT")
                    nc.vector.tensor_copy(out=ynoT[:, :],
                                          in_=_bf(ps_ep[:, 320:384]))
                    nc.tensor.matmul(ps_ep[:, 128:256], ynoT[:, :],
                                     wt["wno2"][:, :], start=True, stop=False)
                    nc.tensor.matmul(ps_ep[:, 128:256], ones1[:, :],
                                     wt["bno2"][:, :], start=False, stop=True)
                    outt = bo.tile([128, 128], F32, tag="outt")
                    nc.vector.tensor_tensor(out=outt[:, :],
                                            in0=ps_ep[:, 128:256],
                                            in1=hrows[:, b, :], op=OP.add)
                    nc.sync.dma_start(
                        out=out_d[b * 128 : (b + 1) * 128, :], in_=outt[:, :])
                    if b + 1 < NBLK:
                        while stw_done < 4:
                            build_stw(nxt, stw_done)
                            stw_done += 1
                    cur = nxt

    _split_multiwait_drains(nc)
    return nc


# ---------------------------------------------------------------------------
# entry point
# ---------------------------------------------------------------------------

_CACHE = {}
LAST_RESULT = {}


def kernel(**inputs):
    _install_ntff_hook_shim()
    wts = _prep_weights_v2(inputs)
    if wts is None:
        from kernel_v1_backup import kernel as kernel_v1  # pragma: no cover

        return kernel_v1(**inputs)

    per_core, eb = _prep_inputs_v2(inputs)
    key = ("v2", eb, round(wts["ewb"], 9))
    if key not in _CACHE:
        _CACHE[key] = build_program_v2(eb, wts["ewb"])
    nc = _CACHE[key]

    wt_arrays = {}
    for k, (shp, dt) in WT_SHAPES_V2.items():
        a = np.ascontiguousarray(wts[k])
        wt_arrays[k] = a.astype(NPBF) if dt == BF16 else a.astype(np.float32)
    in_maps = []
    for c in range(NCORES):
        m = dict(per_core[c])
        m.update(wt_arrays)
        in_maps.append(m)

    trace = bool(int(os.environ.get("KERNEL_TRACE", "0")))
    res = run_bass_kernel_spmd(nc, in_maps, list(range(NCORES)), trace=trace)
    LAST_RESULT["res"] = res

    out = np.concatenate([res.results[c]["out"] for c in range(NCORES)],
                         axis=0)
    return np.ascontiguousarray(out[:N]).astype(np.float32)
